# revision 1
# baseline (speedup 1.0000x reference)
"""Batched cosine-similarity matrix (retrieval_knn) on 8 TRN2 NeuronCores.

reference:  out[b, n, m] = <x[b,n,:], y[b,m,:]> / max(||x[b,n]|| * ||y[b,m]||, 1e-8)
shapes:     x, y: [8, 2048, 512] f32  ->  out: [8, 2048, 2048] f32

Sharding: data-parallel over the batch dim — batch b runs on core b.
Each core receives x[b].T and y[b].T (transposed host-side during
sharding so the contraction dim d lands on SBUF partitions; fp32 DMA
transpose doesn't exist on TRN2 and PE-transposing on device would eat
the tensor engine).

Per-core kernel:
  dots  = xT.T @ yT                     (PE, tiled 128x512, k=4x128)
  ssq_x = ones.T @ (xT*xT) via N=1 MMs  -> [128,1] per n-tile (n on partitions)
  ssq_y = ones128.T @ (yT*yT)           -> [128,512] replicated across partitions
  rx = 1/sqrt(ssq_x)  (ACT sqrt + DVE reciprocal; ACT Rsqrt is banned/inaccurate)
  out_tile = (dots * rx[n]) * ry[m]     (single fused DVE scalar_tensor_tensor)
"""

import numpy as np

import concourse.bass as bass
import concourse.bacc as bacc
import concourse.mybir as mybir
import concourse.tile as tile
from concourse import bass_utils as _bu
from concourse.bass_utils import run_bass_kernel_spmd

# NOTE: walrus --enable-ldw-opt=true was tried to dedupe the per-matmul
# weight loads; it removes few LDWs and produces all-zero output for f32r
# matmuls whose LDW got hoisted (the known standalone-LDW f32r HW bug).
# Keep the default (false).

P = 128          # partitions
D = 512          # feature dim (contraction)
N = 2048         # rows of x / y
B = 8            # batch == n_cores
KC = D // P      # 4 k-chunks
NT = N // P      # 16 n-tiles (output partition tiles)
MC = N // 512    # 4 m-chunks (output free chunks, PSUM-bank width)

F32 = mybir.dt.float32

_CACHED = {}
_VARIANT = ""  # debug switches, e.g. "--norm-f32"


def _build_nc(mm_dtype: str = "float32") -> bass.Bass:
    """Build the single-core Bass program (same program runs SPMD on 8 cores)."""
    nc = bacc.Bacc(trn_type="TRN2", target_bir_lowering=False, debug=False)

    xT = nc.dram_tensor("xT", [D, N], F32, kind="ExternalInput").ap()
    yT = nc.dram_tensor("yT", [D, N], F32, kind="ExternalInput").ap()
    out = nc.dram_tensor("out", [N, N], F32, kind="ExternalOutput").ap()

    mmdt = {"float32": F32, "float32r": mybir.dt.float32r}[mm_dtype]

    with tile.TileContext(nc) as tc:
        with (
            tc.tile_pool(name="xin", bufs=1) as xin_pool,
            tc.tile_pool(name="yin", bufs=1) as yin_pool,
            tc.tile_pool(name="sq", bufs=2) as sq_pool,
            tc.tile_pool(name="consts", bufs=1) as const_pool,
            tc.tile_pool(name="norms", bufs=1) as norm_pool,
            tc.tile_pool(name="ostage", bufs=6) as out_pool,
            tc.tile_pool(name="mm_ps", bufs=4, space="PSUM") as mm_ps_pool,
            tc.tile_pool(name="ry_ps", bufs=2, space="PSUM") as ry_ps_pool,
            tc.tile_pool(name="rx_ps", bufs=2, space="PSUM") as rx_ps_pool,
        ):
            sqdt = F32  # norm pipeline stays fp32 (HW-verified; cheap)
            ones = const_pool.tile([P, P], sqdt, name="ones")
            if sqdt is F32:
                nc.vector.memset(ones, 1.0)
            else:
                ones_f = const_pool.tile([P, P], F32, name="ones_f")
                nc.vector.memset(ones_f, 1.0)
                nc.scalar.copy(ones, ones_f)

            # ---- load inputs: 4 chunks of [128, 2048] each, split into
            # [128, 512] DMAs for load/compute overlap granularity.
            # float32r matmul inputs must be pre-rounded to fp32r; gpsimd
            # (SWDGE) DMAs cast+round during the load.
            xt, yt = [], []
            dma_in = nc.sync if mmdt is F32 else nc.gpsimd
            for k in range(KC):
                xk = xin_pool.tile([P, N], mmdt, name=f"xt{k}", tag=f"xt{k}")
                yk = yin_pool.tile([P, N], mmdt, name=f"yt{k}", tag=f"yt{k}")
                for c in range(MC):
                    cs = slice(c * 512, (c + 1) * 512)
                    dma_in.dma_start(out=xk[:, cs], in_=xT[k * P:(k + 1) * P, cs])
                    dma_in.dma_start(out=yk[:, cs], in_=yT[k * P:(k + 1) * P, cs])
                xt.append(xk)
                yt.append(yk)

            # ---- norms ------------------------------------------------
            # squares (ACT engine; DVE is reserved for the epilogue).
            # Output dtype matches the matmul dtype so the norm matmuls can
            # run at f32r speed (producers must round to f32r).
            xsq, ysq = [], []
            for k in range(KC):
                xs = sq_pool.tile([P, N], sqdt, name=f"xsq{k}", tag=f"xsq{k}", bufs=1)
                ys = sq_pool.tile([P, N], sqdt, name=f"ysq{k}", tag=f"ysq{k}", bufs=1)
                nc.scalar.square(xs, xt[k].bitcast(F32))
                nc.scalar.square(ys, yt[k].bitcast(F32))
                xsq.append(xs)
                ysq.append(ys)

            # rx: per-partition layout [128, 16] via N=1 matmuls (column
            # sums of xsq land with n on partitions).  HW-verified path.
            rx_acc = norm_pool.tile([P, NT], F32, name="rx_acc")
            for k in range(KC):
                rx_ps = rx_ps_pool.tile([P, NT], F32, name="rx_ps", tag="rx_ps")
                for t in range(NT):
                    nc.tensor.matmul(
                        rx_ps[:, t:t + 1],
                        lhsT=xsq[k][:, t * P:(t + 1) * P],
                        rhs=ones[:, 0:1],
                        start=True, stop=True,
                    )
                if k == 0:
                    nc.vector.tensor_copy(rx_acc, rx_ps)
                else:
                    nc.vector.tensor_tensor(rx_acc, rx_acc, rx_ps,
                                            mybir.AluOpType.add)
            rx_sqrt = norm_pool.tile([P, NT], F32, name="rx_sqrt")
            nc.scalar.sqrt(rx_sqrt, rx_acc)
            rx = norm_pool.tile([P, NT], F32, name="rx")
            nc.vector.reciprocal(rx, rx_sqrt)

            # ry: ones128.T @ ysq -> [128, 512] replicated column sums; sqrt
            # on ACT (Rsqrt/Reciprocal activations are banned for accuracy),
            # reciprocal on DVE.  HW-verified path.
            ry = norm_pool.tile([P, N], F32, name="ry")
            sny = norm_pool.tile([P, N], F32, name="sny")
            for c in range(MC):
                cs = slice(c * 512, (c + 1) * 512)
                n_ps = ry_ps_pool.tile([P, 512], F32, name="n_ps", tag="n_ps")
                for k in range(KC):
                    nc.tensor.matmul(
                        n_ps, lhsT=ones, rhs=ysq[k][:, cs],
                        start=(k == 0), stop=(k == KC - 1),
                    )
                nc.scalar.sqrt(sny[:, cs], n_ps)
                nc.vector.reciprocal(ry[:, cs], sny[:, cs])

            # ---- main matmuls + fused epilogue ------------------------
            # k-inner accumulation into one PSUM bank per output tile; the
            # bufs=4 pool rotation keeps PE filling bank n+1 while the DVE
            # epilogue drains bank n.
            for t in range(NT):
                ts_ = slice(t * P, (t + 1) * P)
                for c in range(MC):
                    cs = slice(c * 512, (c + 1) * 512)
                    ps = mm_ps_pool.tile([P, 512], F32, name="ps", tag="ps")
                    for k in range(KC):
                        nc.tensor.matmul(
                            ps, lhsT=xt[k][:, ts_], rhs=yt[k][:, cs],
                            start=(k == 0), stop=(k == KC - 1),
                        )
                    ot = out_pool.tile([P, 512], F32, name="ot", tag="ot")
                    # ot = (ps * rx[:, t]) * ry[:, m-chunk]
                    nc.vector.scalar_tensor_tensor(
                        ot, in0=ps, scalar=rx[:, t:t + 1], in1=ry[:, cs],
                        op0=mybir.AluOpType.mult, op1=mybir.AluOpType.mult,
                    )
                    nc.sync.dma_start(out=out[ts_, cs], in_=ot)

    nc.compile()
    return nc


def _get_nc(mm_dtype: str = "float32") -> bass.Bass:
    if mm_dtype not in _CACHED:
        _CACHED[mm_dtype] = _build_nc(mm_dtype)
    return _CACHED[mm_dtype]


def _shard(x: np.ndarray, y: np.ndarray):
    """Host-side sharding: batch b -> core b, transposed to [512, 2048]."""
    x = np.asarray(x, dtype=np.float32)
    y = np.asarray(y, dtype=np.float32)
    xTs = np.ascontiguousarray(np.transpose(x, (0, 2, 1)))
    yTs = np.ascontiguousarray(np.transpose(y, (0, 2, 1)))
    return [{"xT": xTs[b], "yT": yTs[b]} for b in range(B)]


def _run(x: np.ndarray, y: np.ndarray, mm_dtype: str = "float32",
         trace: bool = False):
    """Returns (out [8, 2048, 2048] f32, BassKernelResults)."""
    nc = _get_nc(mm_dtype)
    in_maps = _shard(x, y)
    res = run_bass_kernel_spmd(nc, in_maps, core_ids=list(range(B)), trace=trace)
    out = np.stack([res.results[b]["out"] for b in range(B)])
    return out, res


def kernel(x: np.ndarray, y: np.ndarray) -> np.ndarray:
    out, _ = _run(x, y, mm_dtype="float32r")
    return out



# revision 3
# speedup vs baseline: 1.3893x; 1.3893x over previous
"""Batched cosine-similarity matrix (retrieval_knn) on 8 TRN2 NeuronCores.

reference:  out[b, n, m] = <x[b,n,:], y[b,m,:]> / max(||x[b,n]|| * ||y[b,m]||, 1e-8)
shapes:     x, y: [8, 2048, 512] f32  ->  out: [8, 2048, 2048] f32

Sharding: data-parallel over the batch dim -- batch b runs on core b.
Each core receives x[b].T and y[b].T in bf16 (cast + transposed host-side;
bf16 rounding of inputs/outputs costs ~4e-3 max-rel error vs the 2e-2 gate).

v2 design notes (vs the f32r baseline at 132.4us):
  * bf16 matmuls: FWL fast-weight-load + the PE's LDW pull-ahead hide the
    per-matmul weight reload that cost ~190ns each at f32r.
  * ~14 warm-up matmuls on a constant tile run during the input-DMA window
    so the PE_HAM clock gate (cold = 1.2 GHz, warm = 2.4 GHz) is already
    released when the real matmuls start.  The f32r baseline ran most of
    its matmuls at 427ns (1.2 GHz) instead of 227ns.
  * bf16 inputs halve the input DMA (8.4 -> 4.2 MB/core); bf16 output
    halves the output DMA (16.8 -> 8.4 MB/core); host upcasts.
  * epilogue (PSUM -> SBUF with per-row rx and per-col ry scaling) is
    split: even t-rows on DVE (one fused scalar_tensor_tensor), odd
    t-rows (t>=5) as ACT scale-copy (x rx) + GpSimd tensor-mult (x ry),
    so no single engine gates the PE.
  * x-squares on GpSimd, y-squares on ACT; 1/sqrt via ACT sqrt +
    DVE reciprocal_approx_fast (the exact DVE reciprocal is 6.4 cyc/elem).
  * emission order is hand-scheduled so every in-order engine queue
    tracks the DMA arrival order (x_c0, y_c0..c3, x_c1..c3).
"""

import numpy as np

import concourse.bass as bass
import concourse.bacc as bacc
import concourse.mybir as mybir
import concourse.tile as tile
from concourse.bass_utils import run_bass_kernel_spmd

P = 128          # partitions
D = 512          # feature dim (contraction)
N = 2048         # rows of x / y
B = 8            # batch == n_cores
KC = D // P      # 4 k-chunks
NT = N // P      # 16 n-tiles (output partition tiles)
MC = N // 512    # 4 m-chunks (output free chunks, PSUM-bank width)
NG = 4           # rx groups (4 t-tiles each, one per x column-chunk)
WARM = 14        # HAM warm-up matmuls

F32 = mybir.dt.float32
BF16 = mybir.dt.bfloat16
BF16_NP = mybir.dt.np(mybir.dt.bfloat16)

_CACHED = {}


def _build_nc() -> bass.Bass:
    """Build the single-core Bass program (same program runs SPMD on 8 cores)."""
    nc = bacc.Bacc(trn_type="TRN2", target_bir_lowering=False, debug=False)

    xT = nc.dram_tensor("xT", [D, N], BF16, kind="ExternalInput").ap()
    yT = nc.dram_tensor("yT", [D, N], BF16, kind="ExternalInput").ap()
    out = nc.dram_tensor("out", [N, N], BF16, kind="ExternalOutput").ap()

    mul = mybir.AluOpType.mult

    with tile.TileContext(nc) as tc:
        with (
            tc.tile_pool(name="xin", bufs=1) as xin_pool,
            tc.tile_pool(name="yin", bufs=1) as yin_pool,
            tc.tile_pool(name="sq", bufs=1) as sq_pool,
            tc.tile_pool(name="consts", bufs=1) as const_pool,
            tc.tile_pool(name="norms", bufs=1) as norm_pool,
            tc.tile_pool(name="ostage", bufs=8) as out_pool,
            tc.tile_pool(name="tmp1", bufs=3) as tmp_pool,
            tc.tile_pool(name="mm_ps", bufs=5, space="PSUM") as mm_ps_pool,
            tc.tile_pool(name="ry_ps", bufs=2, space="PSUM") as ry_ps_pool,
            tc.tile_pool(name="rx_ps", bufs=1, space="PSUM") as rx_ps_pool,
        ):
            # ---- constants --------------------------------------------
            ones_f = const_pool.tile([P, 512], F32, name="ones_f")
            nc.vector.memset(ones_f, 1.0)
            ones_b = const_pool.tile([P, 512], BF16, name="ones_b")
            nc.scalar.copy(ones_b, ones_f)

            # ---- HAM warm-up: keep the PE busy while inputs stream in
            for i in range(WARM):
                wp = mm_ps_pool.tile([P, 512], F32, name="warm", tag="ps")
                nc.tensor.matmul(wp, lhsT=ones_b[:, 0:P], rhs=ones_b,
                                 start=True, stop=True)

            # ---- input DMAs (sync/HWDGE), arrival-priority order ------
            # x chunk c0 first (t0 row lhsT), then all of y (every t-row
            # streams all 4 y chunks), then x c1..c3.
            xt = [xin_pool.tile([P, N], BF16, name=f"xt{k}", tag=f"xt{k}")
                  for k in range(KC)]
            yt = [yin_pool.tile([P, N], BF16, name=f"yt{k}", tag=f"yt{k}")
                  for k in range(KC)]

            def load(dst, src, c):
                cs = slice(c * 512, (c + 1) * 512)
                for k in range(KC):
                    nc.sync.dma_start(out=dst[k][:, cs],
                                      in_=src[k * P:(k + 1) * P, cs])

            load(xt, xT, 0)
            for c in range(MC):
                load(yt, yT, c)
            for c in range(1, MC):
                load(xt, xT, c)

            # ---- squares: y on ACT, x on GpSimd -----------------------
            xsq = [sq_pool.tile([P, N], BF16, name=f"xsq{k}", tag=f"xsq{k}")
                   for k in range(KC)]
            ysq = [sq_pool.tile([P, N], BF16, name=f"ysq{k}", tag=f"ysq{k}")
                   for k in range(KC)]

            def squares_y(c):
                cs = slice(c * 512, (c + 1) * 512)
                for k in range(KC):
                    nc.scalar.square(ysq[k][:, cs], yt[k][:, cs])

            def squares_x(c):
                cs = slice(c * 512, (c + 1) * 512)
                for k in range(KC):
                    nc.gpsimd.tensor_tensor(xsq[k][:, cs], xt[k][:, cs],
                                            xt[k][:, cs], mul)

            # ---- norm tensors -----------------------------------------
            sny = norm_pool.tile([P, N], F32, name="sny")
            ry = norm_pool.tile([P, N], F32, name="ry")
            ry_b = norm_pool.tile([P, N], BF16, name="ry_b")
            rx_sqrt = norm_pool.tile([P, NT], F32, name="rx_sqrt")
            rx = norm_pool.tile([P, NT], F32, name="rx")

            def ry_chunk(c):
                # ry[:, cs] = 1/sqrt(sum_d ysq) -- replicated across partitions
                cs = slice(c * 512, (c + 1) * 512)
                n_ps = ry_ps_pool.tile([P, 512], F32, name="n_ps", tag="n_ps")
                for k in range(KC):
                    nc.tensor.matmul(n_ps, lhsT=ones_b[:, 0:P],
                                     rhs=ysq[k][:, cs],
                                     start=(k == 0), stop=(k == KC - 1))
                nc.scalar.sqrt(sny[:, cs], n_ps)
                nc.vector.reciprocal_approx_fast(ry[:, cs], sny[:, cs])

            def rx_group(g):
                # rx[:, 4g:4g+4] = 1/sqrt(col-sums of xsq t-tiles 4g..4g+3)
                gs = slice(4 * g, 4 * g + 4)
                r_ps = rx_ps_pool.tile([P, NG], F32, name="r_ps", tag="r_ps")
                for tt in range(4):
                    t = 4 * g + tt
                    for k in range(KC):
                        nc.tensor.matmul(
                            r_ps[:, tt:tt + 1],
                            lhsT=xsq[k][:, t * P:(t + 1) * P],
                            rhs=ones_b[:, 0:1],
                            start=(k == 0), stop=(k == KC - 1),
                            skip_group_check=True,
                        )
                nc.scalar.sqrt(rx_sqrt[:, gs], r_ps)
                nc.vector.reciprocal_approx_fast(rx[:, gs], rx_sqrt[:, gs])

            def main_row_mms(t):
                ts_ = slice(t * P, (t + 1) * P)
                row = []
                for c in range(MC):
                    cs = slice(c * 512, (c + 1) * 512)
                    ps = mm_ps_pool.tile([P, 512], F32, name="ps", tag="ps")
                    for k in range(KC):
                        nc.tensor.matmul(ps, lhsT=xt[k][:, ts_],
                                         rhs=yt[k][:, cs],
                                         start=(k == 0), stop=(k == KC - 1))
                    row.append(ps)
                return row

            def epi_dve(t, c, ps):
                ts_ = slice(t * P, (t + 1) * P)
                cs = slice(c * 512, (c + 1) * 512)
                ot = out_pool.tile([P, 512], BF16, name="ot", tag="ot")
                nc.vector.scalar_tensor_tensor(
                    ot, in0=ps, scalar=rx[:, t:t + 1], in1=ry[:, cs],
                    op0=mul, op1=mul,
                )
                nc.sync.dma_start(out=out[ts_, cs], in_=ot)

            def epi_act_gp(t, c, ps):
                ts_ = slice(t * P, (t + 1) * P)
                cs = slice(c * 512, (c + 1) * 512)
                tmp = tmp_pool.tile([P, 512], BF16, name="tmp", tag="tmp")
                nc.scalar.activation(tmp, ps, mybir.ActivationFunctionType.Copy,
                                     scale=rx[:, t:t + 1])
                ot = out_pool.tile([P, 512], BF16, name="ot", tag="ot")
                nc.gpsimd.tensor_tensor(ot, tmp, ry_b[:, cs], mul)
                nc.sync.dma_start(out=out[ts_, cs], in_=ot)

            # ---- hand-scheduled prologue ------------------------------
            squares_y(0)
            squares_x(0)
            ry_chunk(0)
            squares_y(1)
            squares_y(2)
            squares_y(3)
            squares_x(1)

            row0 = main_row_mms(0)
            rx_group(0)
            ry_chunk(1)
            ry_chunk(2)
            ry_chunk(3)
            for c in range(MC):
                epi_dve(0, c, row0[c])

            squares_x(2)
            row1 = main_row_mms(1)
            for c in range(MC):
                epi_dve(1, c, row1[c])
            rx_group(1)

            squares_x(3)
            row2 = main_row_mms(2)
            for c in range(MC):
                epi_dve(2, c, row2[c])
            rx_group(2)

            # ry in bf16 for the GpSimd epilogue path
            for c in range(MC):
                cs = slice(c * 512, (c + 1) * 512)
                nc.gpsimd.tensor_copy(ry_b[:, cs], ry[:, cs])

            row3 = main_row_mms(3)
            for c in range(MC):
                epi_dve(3, c, row3[c])
            rx_group(3)

            # ---- steady state ----------------------------------------
            for t in range(4, NT):
                row = main_row_mms(t)
                for c in range(MC):
                    if t % 2 == 0:
                        epi_dve(t, c, row[c])
                    else:
                        epi_act_gp(t, c, row[c])

    nc.compile()
    return nc


def _get_nc(mm_dtype: str = "bfloat16") -> bass.Bass:
    if mm_dtype not in _CACHED:
        _CACHED[mm_dtype] = _build_nc()
    return _CACHED[mm_dtype]


def _shard(x: np.ndarray, y: np.ndarray):
    """Host-side sharding: batch b -> core b, bf16, transposed to [512, 2048]."""
    x = np.asarray(x, dtype=np.float32)
    y = np.asarray(y, dtype=np.float32)
    xTs = np.ascontiguousarray(np.transpose(x, (0, 2, 1))).astype(BF16_NP)
    yTs = np.ascontiguousarray(np.transpose(y, (0, 2, 1))).astype(BF16_NP)
    return [{"xT": xTs[b], "yT": yTs[b]} for b in range(B)]


def _run(x: np.ndarray, y: np.ndarray, mm_dtype: str = "bfloat16",
         trace: bool = False):
    """Returns (out [8, 2048, 2048] f32, BassKernelResults)."""
    nc = _get_nc(mm_dtype)
    in_maps = _shard(x, y)
    res = run_bass_kernel_spmd(nc, in_maps, core_ids=list(range(B)), trace=trace)
    out = np.stack([res.results[b]["out"].astype(np.float32) for b in range(B)])
    return out, res


def kernel(x: np.ndarray, y: np.ndarray) -> np.ndarray:
    out, _ = _run(x, y)
    return out


# revision 10
# speedup vs baseline: 1.4563x; 1.0483x over previous
"""Batched cosine-similarity matrix (retrieval_knn) on 8 TRN2 NeuronCores.

reference:  out[b, n, m] = <x[b,n,:], y[b,m,:]> / max(||x[b,n]|| * ||y[b,m]||, 1e-8)
shapes:     x, y: [8, 2048, 512] f32  ->  out: [8, 2048, 2048] f32

Sharding: data-parallel over the batch dim -- batch b runs on core b.
Each core receives x[b].T and y[b].T in bf16 (cast + transposed host-side;
bf16 rounding of inputs/outputs costs ~4e-3 max-rel error vs the 2e-2 gate).

v2 design notes (vs the f32r baseline at 132.4us):
  * bf16 matmuls: FWL fast-weight-load + the PE's LDW pull-ahead hide the
    per-matmul weight reload that cost ~190ns each at f32r.
  * ~14 warm-up matmuls on a constant tile run during the input-DMA window
    so the PE_HAM clock gate (cold = 1.2 GHz, warm = 2.4 GHz) is already
    released when the real matmuls start.  The f32r baseline ran most of
    its matmuls at 427ns (1.2 GHz) instead of 227ns.
  * bf16 inputs halve the input DMA (8.4 -> 4.2 MB/core); bf16 output
    halves the output DMA (16.8 -> 8.4 MB/core); host upcasts.
  * epilogue (PSUM -> SBUF with per-row rx and per-col ry scaling) is
    split: even t-rows on DVE (one fused scalar_tensor_tensor), odd
    t-rows (t>=5) as ACT scale-copy (x rx) + GpSimd tensor-mult (x ry),
    so no single engine gates the PE.
  * x-squares on GpSimd, y-squares on ACT; 1/sqrt via ACT sqrt +
    DVE reciprocal_approx_fast (the exact DVE reciprocal is 6.4 cyc/elem).
  * emission order is hand-scheduled so every in-order engine queue
    tracks the DMA arrival order (x_c0, y_c0..c3, x_c1..c3).
"""

import numpy as np

import concourse.bass as bass
import concourse.bacc as bacc
import concourse.mybir as mybir
import concourse.tile as tile
from concourse.bass_utils import run_bass_kernel_spmd

P = 128          # partitions
D = 512          # feature dim (contraction)
N = 2048         # rows of x / y
B = 8            # batch == n_cores
KC = D // P      # 4 k-chunks
NT = N // P      # 16 n-tiles (output partition tiles)
MC = N // 512    # 4 m-chunks (output free chunks, PSUM-bank width)
NG = 4           # rx groups (4 t-tiles each, one per x column-chunk)
WARM = 24        # HAM warm-up matmuls ([128,256], ~214ns each cold)

F32 = mybir.dt.float32
BF16 = mybir.dt.bfloat16
BF16_NP = mybir.dt.np(mybir.dt.bfloat16)

_CACHED = {}


def _build_nc() -> bass.Bass:
    """Build the single-core Bass program (same program runs SPMD on 8 cores)."""
    nc = bacc.Bacc(trn_type="TRN2", target_bir_lowering=False, debug=False)

    xT = nc.dram_tensor("xT", [D, N], BF16, kind="ExternalInput").ap()
    yT = nc.dram_tensor("yT", [D, N], BF16, kind="ExternalInput").ap()
    out = nc.dram_tensor("out", [N, N], BF16, kind="ExternalOutput").ap()

    mul = mybir.AluOpType.mult
    COPY_FN = mybir.ActivationFunctionType.Copy

    with tile.TileContext(nc) as tc:
        with (
            tc.tile_pool(name="xin", bufs=1) as xin_pool,
            tc.tile_pool(name="yin", bufs=1) as yin_pool,
            tc.tile_pool(name="sq", bufs=1) as sq_pool,
            tc.tile_pool(name="consts", bufs=1) as const_pool,
            tc.tile_pool(name="norms", bufs=1) as norm_pool,
            tc.tile_pool(name="ostage", bufs=3) as out_pool,
            tc.tile_pool(name="tmp1", bufs=3) as tmp_pool,
            tc.tile_pool(name="mm_ps", bufs=6, space="PSUM") as mm_ps_pool,
            tc.tile_pool(name="norm_ps", bufs=2, space="PSUM") as norm_ps_pool,
        ):
            # ---- constants --------------------------------------------
            # junk feeds the warm-up matmuls; memset first so the PE
            # dummies start as soon as possible after the preamble.
            junk = const_pool.tile([P, 256], BF16, name="junk")
            nc.vector.memset(junk, 1.0)
            ones_b = const_pool.tile([P, 512], BF16, name="ones_b")
            nc.vector.memset(ones_b, 1.0)

            # ---- HAM warm-up: keep the PE busy while inputs stream in
            for i in range(WARM):
                wp = mm_ps_pool.tile([P, 256], F32, name="warm", tag="ps")
                nc.tensor.matmul(wp, lhsT=junk[:, 0:P], rhs=junk,
                                 start=True, stop=True)

            # ---- input DMAs (sync/HWDGE), arrival-priority order ------
            # x cols 0:512 (t0-t3 lhsT) first, then all of y in halves
            # (every t-row streams all four y chunks), then the rest of x
            # as one wide DMA per k (3KB rows; only needed from row 4 on).
            xt = [xin_pool.tile([P, N], BF16, name=f"xt{k}", tag=f"xt{k}")
                  for k in range(KC)]
            yt = [yin_pool.tile([P, N], BF16, name=f"yt{k}", tag=f"yt{k}")
                  for k in range(KC)]

            for k in range(KC):
                nc.sync.dma_start(out=xt[k][:, 0:512],
                                  in_=xT[k * P:(k + 1) * P, 0:512])
            for h in range(2):
                hs = slice(h * 1024, (h + 1) * 1024)
                for k in range(KC):
                    nc.sync.dma_start(out=yt[k][:, hs],
                                      in_=yT[k * P:(k + 1) * P, hs])
            for k in range(KC):
                nc.sync.dma_start(out=xt[k][:, 512:N],
                                  in_=xT[k * P:(k + 1) * P, 512:N])

            # ---- squares: split across GpSimd / ACT / DVE so the ry/rx
            # chains are ready when the in-order PE queue reaches them.
            xsq = [sq_pool.tile([P, N], BF16, name=f"xsq{k}", tag=f"xsq{k}")
                   for k in range(KC)]
            ysq = [sq_pool.tile([P, N], BF16, name=f"ysq{k}", tag=f"ysq{k}")
                   for k in range(KC)]

            def squares(eng, sq, t_in, c):
                cs = slice(c * 512, (c + 1) * 512)
                for k in range(KC):
                    if eng is nc.scalar:
                        eng.square(sq[k][:, cs], t_in[k][:, cs])
                    else:
                        eng.tensor_tensor(sq[k][:, cs], t_in[k][:, cs],
                                          t_in[k][:, cs], mul)

            # ---- norm tensors -----------------------------------------
            sny = norm_pool.tile([P, N], F32, name="sny")
            ry = norm_pool.tile([P, N], F32, name="ry")
            ry_b = norm_pool.tile([P, N], BF16, name="ry_b")
            rx_sqrt = norm_pool.tile([P, NT], F32, name="rx_sqrt")
            rx = norm_pool.tile([P, NT], F32, name="rx")

            def ry_mms(c):
                cs = slice(c * 512, (c + 1) * 512)
                n_ps = norm_ps_pool.tile([P, 512], F32, name="n_ps", tag="n_ps")
                for k in range(KC):
                    nc.tensor.matmul(n_ps, lhsT=ones_b[:, 0:P],
                                     rhs=ysq[k][:, cs],
                                     start=(k == 0), stop=(k == KC - 1))
                return n_ps

            def ry_finish(c, n_ps):
                cs = slice(c * 512, (c + 1) * 512)
                nc.scalar.sqrt(sny[:, cs], n_ps)
                nc.vector.reciprocal_approx_fast(ry[:, cs], sny[:, cs])

            def rx_group(g):
                # rx[:, 4g:4g+4] = 1/sqrt(col-sums of xsq t-tiles 4g..4g+3)
                gs = slice(4 * g, 4 * g + 4)
                r_ps = norm_ps_pool.tile([P, NG], F32, name="r_ps", tag="n_ps")
                for tt in range(4):
                    t = 4 * g + tt
                    for k in range(KC):
                        nc.tensor.matmul(
                            r_ps[:, tt:tt + 1],
                            lhsT=xsq[k][:, t * P:(t + 1) * P],
                            rhs=ones_b[:, 0:1],
                            start=(k == 0), stop=(k == KC - 1),
                            skip_group_check=True,
                        )
                nc.scalar.sqrt(rx_sqrt[:, gs], r_ps)
                nc.vector.reciprocal_approx_fast(rx[:, gs], rx_sqrt[:, gs])

            def tile_mms(t, c):
                ts_ = slice(t * P, (t + 1) * P)
                cs = slice(c * 512, (c + 1) * 512)
                ps = mm_ps_pool.tile([P, 512], F32, name="ps", tag="ps")
                for k in range(KC):
                    nc.tensor.matmul(ps, lhsT=xt[k][:, ts_],
                                     rhs=yt[k][:, cs],
                                     start=(k == 0), stop=(k == KC - 1))
                return ps

            def epi_dve(t, c, ps, ot):
                cs = slice(c * 512, (c + 1) * 512)
                nc.vector.scalar_tensor_tensor(
                    ot[:, cs], in0=ps, scalar=rx[:, t:t + 1], in1=ry[:, cs],
                    op0=mul, op1=mul,
                )

            def epi_act_gp(t, c, ps, ot):
                cs = slice(c * 512, (c + 1) * 512)
                tmp = tmp_pool.tile([P, 512], BF16, name="tmp", tag="tmp")
                nc.scalar.activation(tmp, ps, COPY_FN, scale=rx[:, t:t + 1])
                nc.gpsimd.tensor_tensor(ot[:, cs], tmp, ry_b[:, cs], mul)

            ACT_GP_ROWS = {5, 7, 9, 11, 13}

            def full_row(t):
                # 16 matmuls, 4 epilogues, one row-wide output DMA issued
                # from the DVE queue (sync is saturated by input issues).
                pss = [tile_mms(t, c) for c in range(MC)]
                ot = out_pool.tile([P, N], BF16, name="ot", tag="ot")
                for c in range(MC):
                    if t in ACT_GP_ROWS:
                        epi_act_gp(t, c, pss[c], ot)
                    else:
                        epi_dve(t, c, pss[c], ot)
                ts_ = slice(t * P, (t + 1) * P)
                nc.scalar.dma_start(out=out[ts_, :], in_=ot)

            # ---- hand-scheduled prologue ------------------------------
            squares(nc.gpsimd, xsq, xt, 0)     # GpSimd: xsq c0
            squares(nc.scalar, ysq, yt, 0)     # ACT:    ysq c0
            squares(nc.scalar, ysq, yt, 1)     # ACT:    ysq c1
            squares(nc.vector, ysq, yt, 2)     # DVE:    ysq c2
            squares(nc.scalar, ysq, yt, 3)     # ACT:    ysq c3

            ps00 = tile_mms(0, 0)
            ps01 = tile_mms(0, 1)
            rx_group(0)
            nps0 = ry_mms(0)
            ry_finish(0, nps0)
            ps02 = tile_mms(0, 2)
            ps03 = tile_mms(0, 3)
            nps = ry_mms(1)
            ry_finish(1, nps)
            nps = ry_mms(2)
            ry_finish(2, nps)
            nps = ry_mms(3)
            ry_finish(3, nps)

            ot0 = out_pool.tile([P, N], BF16, name="ot", tag="ot")
            for c, ps in enumerate([ps00, ps01, ps02, ps03]):
                epi_dve(0, c, ps, ot0)
            nc.scalar.dma_start(out=out[0:P, :], in_=ot0)

            full_row(1)
            full_row(2)
            squares(nc.gpsimd, xsq, xt, 1)     # GpSimd: xsq c1
            full_row(3)
            rx_group(1)
            squares(nc.gpsimd, xsq, xt, 2)     # GpSimd: xsq c2
            squares(nc.gpsimd, xsq, xt, 3)     # GpSimd: xsq c3
            # ry in bf16 for the GpSimd epilogue path
            for c in range(MC):
                cs = slice(c * 512, (c + 1) * 512)
                nc.gpsimd.tensor_copy(ry_b[:, cs], ry[:, cs])
            full_row(4)
            full_row(5)
            full_row(6)
            rx_group(2)
            full_row(7)
            full_row(8)
            full_row(9)
            rx_group(3)
            for t in range(10, NT - 1):
                full_row(t)

            # last row: alternate engines per tile + two half-row DMAs so
            # the drain tail after the final matmul is as short as possible.
            t = NT - 1
            pss = [tile_mms(t, c) for c in range(MC)]
            ot = out_pool.tile([P, N], BF16, name="ot", tag="ot")
            epi_dve(t, 0, pss[0], ot)
            epi_act_gp(t, 1, pss[1], ot)
            nc.scalar.dma_start(out=out[t * P:(t + 1) * P, 0:1024],
                                in_=ot[:, 0:1024])
            epi_dve(t, 2, pss[2], ot)
            epi_act_gp(t, 3, pss[3], ot)
            nc.scalar.dma_start(out=out[t * P:(t + 1) * P, 1024:N],
                                in_=ot[:, 1024:N])

    nc.compile()
    return nc


def _get_nc(mm_dtype: str = "bfloat16") -> bass.Bass:
    if mm_dtype not in _CACHED:
        _CACHED[mm_dtype] = _build_nc()
    return _CACHED[mm_dtype]


def _shard(x: np.ndarray, y: np.ndarray):
    """Host-side sharding: batch b -> core b, bf16, transposed to [512, 2048]."""
    x = np.asarray(x, dtype=np.float32)
    y = np.asarray(y, dtype=np.float32)
    xTs = np.ascontiguousarray(np.transpose(x, (0, 2, 1))).astype(BF16_NP)
    yTs = np.ascontiguousarray(np.transpose(y, (0, 2, 1))).astype(BF16_NP)
    return [{"xT": xTs[b], "yT": yTs[b]} for b in range(B)]


def _run(x: np.ndarray, y: np.ndarray, mm_dtype: str = "bfloat16",
         trace: bool = False):
    """Returns (out [8, 2048, 2048] f32, BassKernelResults)."""
    nc = _get_nc(mm_dtype)
    in_maps = _shard(x, y)
    res = run_bass_kernel_spmd(nc, in_maps, core_ids=list(range(B)), trace=trace)
    out = np.stack([res.results[b]["out"].astype(np.float32) for b in range(B)])
    return out, res


def kernel(x: np.ndarray, y: np.ndarray) -> np.ndarray:
    out, _ = _run(x, y)
    return out


# revision 14
# speedup vs baseline: 1.5800x; 1.0849x over previous
"""Batched cosine-similarity matrix (retrieval_knn) on 8 TRN2 NeuronCores.

reference:  out[b, n, m] = <x[b,n,:], y[b,m,:]> / max(||x[b,n]|| * ||y[b,m]||, 1e-8)
shapes:     x, y: [8, 2048, 512] f32  ->  out: [8, 2048, 2048] f32

Sharding: data-parallel over the batch dim -- batch b runs on core b.
Each core receives x[b].T and y[b].T in bf16 (cast + transposed host-side;
bf16 rounding of inputs/outputs costs ~4e-3 max-rel error vs the 2e-2 gate).

v2 design notes (vs the f32r baseline at 132.4us):
  * bf16 matmuls: FWL fast-weight-load + the PE's LDW pull-ahead hide the
    per-matmul weight reload that cost ~190ns each at f32r.
  * ~14 warm-up matmuls on a constant tile run during the input-DMA window
    so the PE_HAM clock gate (cold = 1.2 GHz, warm = 2.4 GHz) is already
    released when the real matmuls start.  The f32r baseline ran most of
    its matmuls at 427ns (1.2 GHz) instead of 227ns.
  * bf16 inputs halve the input DMA (8.4 -> 4.2 MB/core); bf16 output
    halves the output DMA (16.8 -> 8.4 MB/core); host upcasts.
  * epilogue (PSUM -> SBUF with per-row rx and per-col ry scaling) is
    split: even t-rows on DVE (one fused scalar_tensor_tensor), odd
    t-rows (t>=5) as ACT scale-copy (x rx) + GpSimd tensor-mult (x ry),
    so no single engine gates the PE.
  * x-squares on GpSimd, y-squares on ACT; 1/sqrt via ACT sqrt +
    DVE reciprocal_approx_fast (the exact DVE reciprocal is 6.4 cyc/elem).
  * emission order is hand-scheduled so every in-order engine queue
    tracks the DMA arrival order (x_c0, y_c0..c3, x_c1..c3).
"""

import numpy as np

import concourse.bass as bass
import concourse.bacc as bacc
import concourse.mybir as mybir
import concourse.tile as tile
from concourse.bass_utils import run_bass_kernel_spmd

P = 128          # partitions
D = 512          # feature dim (contraction)
N = 2048         # rows of x / y
B = 8            # batch == n_cores
KC = D // P      # 4 k-chunks
NT = N // P      # 16 n-tiles (output partition tiles)
MC = N // 512    # 4 m-chunks (output free chunks, PSUM-bank width)
NG = 4           # rx groups (4 t-tiles each, one per x column-chunk)
WARM = 24        # HAM warm-up matmuls ([128,256], ~214ns each cold)

F32 = mybir.dt.float32
BF16 = mybir.dt.bfloat16
BF16_NP = mybir.dt.np(mybir.dt.bfloat16)

_CACHED = {}


def _build_nc() -> bass.Bass:
    """Build the single-core Bass program (same program runs SPMD on 8 cores)."""
    nc = bacc.Bacc(trn_type="TRN2", target_bir_lowering=False, debug=False)

    xT = nc.dram_tensor("xT", [D, N], BF16, kind="ExternalInput").ap()
    yT = nc.dram_tensor("yT", [D, N], BF16, kind="ExternalInput").ap()
    out = nc.dram_tensor("out", [N, N], BF16, kind="ExternalOutput").ap()

    mul = mybir.AluOpType.mult
    COPY_FN = mybir.ActivationFunctionType.Copy

    with tile.TileContext(nc) as tc:
        with (
            tc.tile_pool(name="xin", bufs=1) as xin_pool,
            tc.tile_pool(name="yin", bufs=1) as yin_pool,
            tc.tile_pool(name="sq", bufs=1) as sq_pool,
            tc.tile_pool(name="consts", bufs=1) as const_pool,
            tc.tile_pool(name="norms", bufs=1) as norm_pool,
            tc.tile_pool(name="ostage", bufs=3) as out_pool,
            tc.tile_pool(name="tmp1", bufs=3) as tmp_pool,
            tc.tile_pool(name="mm_ps", bufs=6, space="PSUM") as mm_ps_pool,
            tc.tile_pool(name="norm_ps", bufs=2, space="PSUM") as norm_ps_pool,
        ):
            # ---- constants --------------------------------------------
            # junk feeds the warm-up matmuls; memset first so the PE
            # dummies start as soon as possible after the preamble.
            junk = const_pool.tile([P, 256], BF16, name="junk")
            nc.vector.memset(junk, 1.0)
            ones_b = const_pool.tile([P, 512], BF16, name="ones_b")
            nc.vector.memset(ones_b, 1.0)

            # ---- HAM warm-up: keep the PE busy while inputs stream in
            for i in range(WARM):
                wp = mm_ps_pool.tile([P, 256], F32, name="warm", tag="ps")
                nc.tensor.matmul(wp, lhsT=junk[:, 0:P], rhs=junk,
                                 start=True, stop=True)

            # ---- input DMAs (sync/HWDGE), arrival-priority order ------
            # x cols 0:512 (t0-t3 lhsT) first, then all of y in halves
            # (every t-row streams all four y chunks), then the rest of x
            # as one wide DMA per k (3KB rows; only needed from row 4 on).
            xt = [xin_pool.tile([P, N], BF16, name=f"xt{k}", tag=f"xt{k}")
                  for k in range(KC)]
            yt = [yin_pool.tile([P, N], BF16, name=f"yt{k}", tag=f"yt{k}")
                  for k in range(KC)]

            for k in range(KC):
                nc.sync.dma_start(out=yt[k][:, 0:1024],
                                  in_=yT[k * P:(k + 1) * P, 0:1024])
            for k in range(KC):
                nc.sync.dma_start(out=xt[k][:, 0:512],
                                  in_=xT[k * P:(k + 1) * P, 0:512])
            for k in range(KC):
                nc.sync.dma_start(out=yt[k][:, 1024:N],
                                  in_=yT[k * P:(k + 1) * P, 1024:N])
            for c in range(1, MC):
                cs = slice(c * 512, (c + 1) * 512)
                for k in range(KC):
                    nc.sync.dma_start(out=xt[k][:, cs],
                                      in_=xT[k * P:(k + 1) * P, cs])

            # ---- squares: split across GpSimd / ACT / DVE so the ry/rx
            # chains are ready when the in-order PE queue reaches them.
            xsq = [sq_pool.tile([P, N], BF16, name=f"xsq{k}", tag=f"xsq{k}")
                   for k in range(KC)]
            ysq = [sq_pool.tile([P, N], BF16, name=f"ysq{k}", tag=f"ysq{k}")
                   for k in range(KC)]

            def squares(eng, sq, t_in, c):
                cs = slice(c * 512, (c + 1) * 512)
                for k in range(KC):
                    if eng is nc.scalar:
                        eng.square(sq[k][:, cs], t_in[k][:, cs])
                    else:
                        eng.tensor_tensor(sq[k][:, cs], t_in[k][:, cs],
                                          t_in[k][:, cs], mul)

            # ---- norm tensors -----------------------------------------
            sny = norm_pool.tile([P, N], F32, name="sny")
            ry = norm_pool.tile([P, N], F32, name="ry")
            ry_b = norm_pool.tile([P, N], BF16, name="ry_b")
            rx_sqrt = norm_pool.tile([P, NT], F32, name="rx_sqrt")
            rx = norm_pool.tile([P, NT], F32, name="rx")

            def ry_mms(c):
                cs = slice(c * 512, (c + 1) * 512)
                n_ps = norm_ps_pool.tile([P, 512], F32, name="n_ps", tag="n_ps")
                for k in range(KC):
                    nc.tensor.matmul(n_ps, lhsT=ones_b[:, 0:P],
                                     rhs=ysq[k][:, cs],
                                     start=(k == 0), stop=(k == KC - 1))
                return n_ps

            def ry_finish(c, n_ps):
                cs = slice(c * 512, (c + 1) * 512)
                nc.scalar.sqrt(sny[:, cs], n_ps)
                nc.vector.reciprocal_approx_fast(ry[:, cs], sny[:, cs])

            def rx_group(g):
                # rx[:, 4g:4g+4] = 1/sqrt(col-sums of xsq t-tiles 4g..4g+3)
                gs = slice(4 * g, 4 * g + 4)
                r_ps = norm_ps_pool.tile([P, NG], F32, name="r_ps", tag="n_ps")
                for tt in range(4):
                    t = 4 * g + tt
                    for k in range(KC):
                        nc.tensor.matmul(
                            r_ps[:, tt:tt + 1],
                            lhsT=xsq[k][:, t * P:(t + 1) * P],
                            rhs=ones_b[:, 0:1],
                            start=(k == 0), stop=(k == KC - 1),
                            skip_group_check=True,
                        )
                nc.scalar.sqrt(rx_sqrt[:, gs], r_ps)
                nc.vector.reciprocal_approx_fast(rx[:, gs], rx_sqrt[:, gs])

            def tile_mms(t, c):
                ts_ = slice(t * P, (t + 1) * P)
                cs = slice(c * 512, (c + 1) * 512)
                ps = mm_ps_pool.tile([P, 512], F32, name="ps", tag="ps")
                for k in range(KC):
                    nc.tensor.matmul(ps, lhsT=xt[k][:, ts_],
                                     rhs=yt[k][:, cs],
                                     start=(k == 0), stop=(k == KC - 1))
                return ps

            def epi_dve(t, c, ps, ot):
                cs = slice(c * 512, (c + 1) * 512)
                nc.vector.scalar_tensor_tensor(
                    ot[:, cs], in0=ps, scalar=rx[:, t:t + 1], in1=ry[:, cs],
                    op0=mul, op1=mul,
                )

            def epi_act_gp(t, c, ps, ot):
                cs = slice(c * 512, (c + 1) * 512)
                tmp = tmp_pool.tile([P, 512], BF16, name="tmp", tag="tmp")
                nc.scalar.activation(tmp, ps, COPY_FN, scale=rx[:, t:t + 1])
                nc.gpsimd.tensor_tensor(ot[:, cs], tmp, ry_b[:, cs], mul)

            ACT_GP_ROWS = {5, 7, 9, 11, 13}

            def full_row(t):
                # 16 matmuls, 4 epilogues, one row-wide output DMA issued
                # from the DVE queue (sync is saturated by input issues).
                pss = [tile_mms(t, c) for c in range(MC)]
                ot = out_pool.tile([P, N], BF16, name="ot", tag="ot")
                for c in range(MC):
                    if t in ACT_GP_ROWS:
                        epi_act_gp(t, c, pss[c], ot)
                    else:
                        epi_dve(t, c, pss[c], ot)
                ts_ = slice(t * P, (t + 1) * P)
                nc.scalar.dma_start(out=out[ts_, :], in_=ot)

            # ---- prologue: squares assigned by arrival time and engine
            # speed (DVE 424ns/chunk, ACT 612ns, GpSimd 1007ns).
            squares(nc.vector, xsq, xt, 0)     # DVE:    xsq c0 (rx_g0 feed)
            squares(nc.scalar, ysq, yt, 0)     # ACT:    ysq c0
            squares(nc.scalar, ysq, yt, 1)     # ACT:    ysq c1
            squares(nc.vector, ysq, yt, 2)     # DVE:    ysq c2
            squares(nc.vector, ysq, yt, 3)     # DVE:    ysq c3
            squares(nc.gpsimd, xsq, xt, 1)     # GpSimd: xsq c1 (rx_g1)
            squares(nc.gpsimd, xsq, xt, 2)     # GpSimd: xsq c2 (rx_g2)
            squares(nc.gpsimd, xsq, xt, 3)     # GpSimd: xsq c3 (rx_g3)

            ps00 = tile_mms(0, 0)
            ps01 = tile_mms(0, 1)
            rx_group(0)
            nps0 = ry_mms(0)
            ry_finish(0, nps0)
            ps02 = tile_mms(0, 2)
            ps03 = tile_mms(0, 3)
            nps = ry_mms(1)
            ry_finish(1, nps)
            nps = ry_mms(2)
            ry_finish(2, nps)
            nps = ry_mms(3)
            ry_finish(3, nps)

            ot0 = out_pool.tile([P, N], BF16, name="ot", tag="ot")
            for c, ps in enumerate([ps00, ps01, ps02, ps03]):
                epi_dve(0, c, ps, ot0)
            nc.scalar.dma_start(out=out[0:P, :], in_=ot0)

            full_row(1)
            full_row(2)
            full_row(3)
            rx_group(1)
            # ry in bf16 for the GpSimd epilogue path
            for c in range(MC):
                cs = slice(c * 512, (c + 1) * 512)
                nc.gpsimd.tensor_copy(ry_b[:, cs], ry[:, cs])
            full_row(4)
            full_row(5)
            full_row(6)
            rx_group(2)
            full_row(7)
            full_row(8)
            full_row(9)
            rx_group(3)
            for t in range(10, NT - 1):
                full_row(t)

            # last row: alternate engines per tile + two half-row DMAs so
            # the drain tail after the final matmul is as short as possible.
            t = NT - 1
            pss = [tile_mms(t, c) for c in range(MC)]
            ot = out_pool.tile([P, N], BF16, name="ot", tag="ot")
            epi_dve(t, 0, pss[0], ot)
            epi_act_gp(t, 1, pss[1], ot)
            nc.scalar.dma_start(out=out[t * P:(t + 1) * P, 0:1024],
                                in_=ot[:, 0:1024])
            epi_dve(t, 2, pss[2], ot)
            epi_act_gp(t, 3, pss[3], ot)
            nc.scalar.dma_start(out=out[t * P:(t + 1) * P, 1024:N],
                                in_=ot[:, 1024:N])

    nc.compile()
    return nc


def _get_nc(mm_dtype: str = "bfloat16") -> bass.Bass:
    if mm_dtype not in _CACHED:
        _CACHED[mm_dtype] = _build_nc()
    return _CACHED[mm_dtype]


def _shard(x: np.ndarray, y: np.ndarray):
    """Host-side sharding: batch b -> core b, bf16, transposed to [512, 2048]."""
    x = np.asarray(x, dtype=np.float32)
    y = np.asarray(y, dtype=np.float32)
    xTs = np.ascontiguousarray(np.transpose(x, (0, 2, 1))).astype(BF16_NP)
    yTs = np.ascontiguousarray(np.transpose(y, (0, 2, 1))).astype(BF16_NP)
    return [{"xT": xTs[b], "yT": yTs[b]} for b in range(B)]


def _run(x: np.ndarray, y: np.ndarray, mm_dtype: str = "bfloat16",
         trace: bool = False):
    """Returns (out [8, 2048, 2048] f32, BassKernelResults)."""
    nc = _get_nc(mm_dtype)
    in_maps = _shard(x, y)
    res = run_bass_kernel_spmd(nc, in_maps, core_ids=list(range(B)), trace=trace)
    out = np.stack([res.results[b]["out"].astype(np.float32) for b in range(B)])
    return out, res


def kernel(x: np.ndarray, y: np.ndarray) -> np.ndarray:
    out, _ = _run(x, y)
    return out


# revision 22
# speedup vs baseline: 1.5875x; 1.0048x over previous
"""Batched cosine-similarity matrix (retrieval_knn) on 8 TRN2 NeuronCores.

reference:  out[b, n, m] = <x[b,n,:], y[b,m,:]> / max(||x[b,n]|| * ||y[b,m]||, 1e-8)
shapes:     x, y: [8, 2048, 512] f32  ->  out: [8, 2048, 2048] f32

Sharding: data-parallel over the batch dim -- batch b runs on core b.
Each core receives x[b].T and y[b].T in bf16 (cast + transposed host-side;
bf16 rounding of inputs/outputs costs ~4e-3 max-rel error vs the 2e-2 gate).

v2 design notes (vs the f32r baseline at 132.4us):
  * bf16 matmuls: FWL fast-weight-load + the PE's LDW pull-ahead hide the
    per-matmul weight reload that cost ~190ns each at f32r.
  * ~14 warm-up matmuls on a constant tile run during the input-DMA window
    so the PE_HAM clock gate (cold = 1.2 GHz, warm = 2.4 GHz) is already
    released when the real matmuls start.  The f32r baseline ran most of
    its matmuls at 427ns (1.2 GHz) instead of 227ns.
  * bf16 inputs halve the input DMA (8.4 -> 4.2 MB/core); bf16 output
    halves the output DMA (16.8 -> 8.4 MB/core); host upcasts.
  * epilogue (PSUM -> SBUF with per-row rx and per-col ry scaling) is
    split: even t-rows on DVE (one fused scalar_tensor_tensor), odd
    t-rows (t>=5) as ACT scale-copy (x rx) + GpSimd tensor-mult (x ry),
    so no single engine gates the PE.
  * x-squares on GpSimd, y-squares on ACT; 1/sqrt via ACT sqrt +
    DVE reciprocal_approx_fast (the exact DVE reciprocal is 6.4 cyc/elem).
  * emission order is hand-scheduled so every in-order engine queue
    tracks the DMA arrival order (x_c0, y_c0..c3, x_c1..c3).
"""

import numpy as np

import concourse.bass as bass
import concourse.bacc as bacc
import concourse.mybir as mybir
import concourse.tile as tile
from concourse.bass_utils import run_bass_kernel_spmd

P = 128          # partitions
D = 512          # feature dim (contraction)
N = 2048         # rows of x / y
B = 8            # batch == n_cores
KC = D // P      # 4 k-chunks
NT = N // P      # 16 n-tiles (output partition tiles)
MC = N // 512    # 4 m-chunks (output free chunks, PSUM-bank width)
NG = 4           # rx groups (4 t-tiles each, one per x column-chunk)
WARM = 22        # HAM warm-up matmuls ([128,256], ~214ns each cold)

F32 = mybir.dt.float32
BF16 = mybir.dt.bfloat16
BF16_NP = mybir.dt.np(mybir.dt.bfloat16)

_CACHED = {}


def _build_nc() -> bass.Bass:
    """Build the single-core Bass program (same program runs SPMD on 8 cores)."""
    nc = bacc.Bacc(trn_type="TRN2", target_bir_lowering=False, debug=False)

    # Packed host layout: xP[p, c*2048 + k*512 + j] = x[c*512+j, k*128+p]
    # (and same for y) so every 512-row chunk is ONE contiguous [128, 2048]
    # DMA with 4KB lines -- 8 input dma_starts total instead of 24.
    xP = nc.dram_tensor("xP", [P, KC * N], BF16, kind="ExternalInput").ap()
    yP = nc.dram_tensor("yP", [P, KC * N], BF16, kind="ExternalInput").ap()
    out = nc.dram_tensor("out", [N, N], BF16, kind="ExternalOutput").ap()

    def xoff(t, k):
        # lhsT column block for output tile-row t, contraction chunk k
        return (t // 4) * 2048 + k * 512 + (t % 4) * P

    def yoff(k, c):
        # rhs column block for output col-chunk c, contraction chunk k
        return c * 2048 + k * 512

    mul = mybir.AluOpType.mult
    COPY_FN = mybir.ActivationFunctionType.Copy

    with tile.TileContext(nc) as tc:
        with (
            tc.tile_pool(name="xin", bufs=1) as xin_pool,
            tc.tile_pool(name="yin", bufs=1) as yin_pool,
            tc.tile_pool(name="sq", bufs=1) as sq_pool,
            tc.tile_pool(name="consts", bufs=1) as const_pool,
            tc.tile_pool(name="norms", bufs=1) as norm_pool,
            tc.tile_pool(name="ostage", bufs=3) as out_pool,
            tc.tile_pool(name="tmp1", bufs=3) as tmp_pool,
            tc.tile_pool(name="mm_ps", bufs=6, space="PSUM") as mm_ps_pool,
            tc.tile_pool(name="norm_ps", bufs=2, space="PSUM") as norm_ps_pool,
        ):
            # ---- constants --------------------------------------------
            # junk feeds the warm-up matmuls; memset first so the PE
            # dummies start as soon as possible after the preamble.
            junk = const_pool.tile([P, 256], BF16, name="junk")
            nc.vector.memset(junk, 1.0)
            ones_b = const_pool.tile([P, 512], BF16, name="ones_b")
            nc.vector.memset(ones_b, 1.0)

            # ---- HAM warm-up: keep the PE busy while inputs stream in
            for i in range(WARM):
                wp = mm_ps_pool.tile([P, 256], F32, name="warm", tag="ps")
                nc.tensor.matmul(wp, lhsT=junk[:, 0:P], rhs=junk,
                                 start=True, stop=True)

            # ---- input DMAs (sync/HWDGE), arrival-priority order ------
            # one contiguous [128, 2048] DMA per 512-column chunk:
            # x chunk c0 (tile-rows 0-3 lhsT), then y chunks c0..c3, then
            # x chunks c1..c3 (only needed from tile-row 4 / 8 / 12 on).
            xt = xin_pool.tile([P, KC * N], BF16, name="xt", tag="xt")
            yt = yin_pool.tile([P, KC * N], BF16, name="yt", tag="yt")

            def cb(c):
                return slice(c * 2048, (c + 1) * 2048)

            nc.sync.dma_start(out=xt[:, cb(0)], in_=xP[:, cb(0)])
            for c in range(MC):
                nc.sync.dma_start(out=yt[:, cb(c)], in_=yP[:, cb(c)])
            for c in range(1, MC):
                nc.sync.dma_start(out=xt[:, cb(c)], in_=xP[:, cb(c)])

            # ---- squares: split across GpSimd / ACT / DVE so the ry/rx
            # chains are ready when the in-order PE queue reaches them.
            xsq = sq_pool.tile([P, KC * N], BF16, name="xsq", tag="xsq")
            ysq = sq_pool.tile([P, KC * N], BF16, name="ysq", tag="ysq")

            def squares(eng, sq, t_in, c):
                if eng is nc.scalar:
                    eng.square(sq[:, cb(c)], t_in[:, cb(c)])
                else:
                    eng.tensor_tensor(sq[:, cb(c)], t_in[:, cb(c)],
                                      t_in[:, cb(c)], mul)

            # ---- norm tensors -----------------------------------------
            sny = norm_pool.tile([P, N], F32, name="sny")
            ry = norm_pool.tile([P, N], F32, name="ry")
            ry_b = norm_pool.tile([P, N], BF16, name="ry_b")
            rx_sqrt = norm_pool.tile([P, NT], F32, name="rx_sqrt")
            rx = norm_pool.tile([P, NT], F32, name="rx")

            def ry_mms(c):
                n_ps = norm_ps_pool.tile([P, 512], F32, name="n_ps", tag="n_ps")
                for k in range(KC):
                    o = yoff(k, c)
                    nc.tensor.matmul(n_ps, lhsT=ones_b[:, 0:P],
                                     rhs=ysq[:, o:o + 512],
                                     start=(k == 0), stop=(k == KC - 1))
                return n_ps

            def ry_finish(c, n_ps):
                cs = slice(c * 512, (c + 1) * 512)
                nc.scalar.sqrt(sny[:, cs], n_ps)
                nc.vector.reciprocal_approx_fast(ry[:, cs], sny[:, cs])

            def rx_group(g):
                # rx[:, 4g:4g+4] = 1/sqrt(col-sums of xsq t-tiles 4g..4g+3)
                gs = slice(4 * g, 4 * g + 4)
                r_ps = norm_ps_pool.tile([P, NG], F32, name="r_ps", tag="n_ps")
                for tt in range(4):
                    t = 4 * g + tt
                    for k in range(KC):
                        o = xoff(t, k)
                        nc.tensor.matmul(
                            r_ps[:, tt:tt + 1],
                            lhsT=xsq[:, o:o + P],
                            rhs=ones_b[:, 0:1],
                            start=(k == 0), stop=(k == KC - 1),
                            skip_group_check=True,
                        )
                nc.scalar.sqrt(rx_sqrt[:, gs], r_ps)
                nc.vector.reciprocal_approx_fast(rx[:, gs], rx_sqrt[:, gs])

            def tile_mms(t, c):
                ps = mm_ps_pool.tile([P, 512], F32, name="ps", tag="ps")
                for k in range(KC):
                    xo = xoff(t, k)
                    yo = yoff(k, c)
                    nc.tensor.matmul(ps, lhsT=xt[:, xo:xo + P],
                                     rhs=yt[:, yo:yo + 512],
                                     start=(k == 0), stop=(k == KC - 1))
                return ps

            def epi_dve(t, c, ps, ot):
                cs = slice(c * 512, (c + 1) * 512)
                nc.vector.scalar_tensor_tensor(
                    ot[:, cs], in0=ps, scalar=rx[:, t:t + 1], in1=ry[:, cs],
                    op0=mul, op1=mul,
                )

            def epi_act_gp(t, c, ps, ot):
                cs = slice(c * 512, (c + 1) * 512)
                tmp = tmp_pool.tile([P, 512], BF16, name="tmp", tag="tmp")
                nc.scalar.activation(tmp, ps, COPY_FN, scale=rx[:, t:t + 1])
                nc.gpsimd.tensor_tensor(ot[:, cs], tmp, ry_b[:, cs], mul)

            ACT_GP_ROWS = {5, 7, 9, 11, 13}

            def full_row(t):
                # 16 matmuls, 4 epilogues, one row-wide output DMA issued
                # from the DVE queue (sync is saturated by input issues).
                pss = [tile_mms(t, c) for c in range(MC)]
                ot = out_pool.tile([P, N], BF16, name="ot", tag="ot")
                for c in range(MC):
                    if t in ACT_GP_ROWS:
                        epi_act_gp(t, c, pss[c], ot)
                    else:
                        epi_dve(t, c, pss[c], ot)
                ts_ = slice(t * P, (t + 1) * P)
                nc.scalar.dma_start(out=out[ts_, :], in_=ot)

            # ---- prologue: squares assigned by arrival time and engine
            # speed (DVE 424ns/chunk, ACT 612ns, GpSimd 1007ns).
            squares(nc.vector, xsq, xt, 0)     # DVE:    xsq c0 (rx_g0 feed)
            squares(nc.scalar, ysq, yt, 0)     # ACT:    ysq c0
            squares(nc.scalar, ysq, yt, 1)     # ACT:    ysq c1
            squares(nc.vector, ysq, yt, 2)     # DVE:    ysq c2
            squares(nc.vector, ysq, yt, 3)     # DVE:    ysq c3
            squares(nc.gpsimd, xsq, xt, 1)     # GpSimd: xsq c1 (rx_g1)
            squares(nc.gpsimd, xsq, xt, 2)     # GpSimd: xsq c2 (rx_g2)
            squares(nc.gpsimd, xsq, xt, 3)     # GpSimd: xsq c3 (rx_g3)

            ps00 = tile_mms(0, 0)
            ps01 = tile_mms(0, 1)
            rx_group(0)
            nps0 = ry_mms(0)
            ry_finish(0, nps0)
            ps02 = tile_mms(0, 2)
            ps03 = tile_mms(0, 3)
            nps = ry_mms(1)
            ry_finish(1, nps)
            nps = ry_mms(2)
            ry_finish(2, nps)
            nps = ry_mms(3)
            ry_finish(3, nps)

            ot0 = out_pool.tile([P, N], BF16, name="ot", tag="ot")
            for c, ps in enumerate([ps00, ps01, ps02, ps03]):
                epi_dve(0, c, ps, ot0)
            nc.scalar.dma_start(out=out[0:P, :], in_=ot0)

            full_row(1)
            full_row(2)
            full_row(3)
            rx_group(1)
            # ry in bf16 for the GpSimd epilogue path
            for c in range(MC):
                cs = slice(c * 512, (c + 1) * 512)
                nc.gpsimd.tensor_copy(ry_b[:, cs], ry[:, cs])
            full_row(4)
            full_row(5)
            full_row(6)
            rx_group(2)
            full_row(7)
            full_row(8)
            full_row(9)
            rx_group(3)
            for t in range(10, NT - 1):
                full_row(t)

            # last row: alternate engines per tile + two half-row DMAs so
            # the drain tail after the final matmul is as short as possible
            # (the last tile's epilogue is the single-pass DVE one).
            t = NT - 1
            pss = [tile_mms(t, c) for c in range(MC)]
            ot = out_pool.tile([P, N], BF16, name="ot", tag="ot")
            epi_dve(t, 0, pss[0], ot)
            epi_act_gp(t, 1, pss[1], ot)
            nc.scalar.dma_start(out=out[t * P:(t + 1) * P, 0:1024],
                                in_=ot[:, 0:1024])
            epi_act_gp(t, 2, pss[2], ot)
            epi_dve(t, 3, pss[3], ot)
            nc.scalar.dma_start(out=out[t * P:(t + 1) * P, 1024:N],
                                in_=ot[:, 1024:N])

    nc.compile()
    return nc


def _get_nc(mm_dtype: str = "bfloat16") -> bass.Bass:
    if mm_dtype not in _CACHED:
        _CACHED[mm_dtype] = _build_nc()
    return _CACHED[mm_dtype]


def _pack(a: np.ndarray) -> np.ndarray:
    """[2048, 512] -> [128, 8192] with layout [p, c*2048 + k*512 + j] =
    a[c*512 + j, k*128 + p], so each 512-row chunk is one contiguous DMA."""
    v = a.reshape(MC, 512, KC, P)            # [c, j, k, p]
    return np.ascontiguousarray(
        v.transpose(3, 0, 2, 1).reshape(P, KC * N)).astype(BF16_NP)


def _shard(x: np.ndarray, y: np.ndarray):
    """Host-side sharding: batch b -> core b, bf16, chunk-packed."""
    x = np.asarray(x, dtype=np.float32)
    y = np.asarray(y, dtype=np.float32)
    return [{"xP": _pack(x[b]), "yP": _pack(y[b])} for b in range(B)]


def _run(x: np.ndarray, y: np.ndarray, mm_dtype: str = "bfloat16",
         trace: bool = False):
    """Returns (out [8, 2048, 2048] f32, BassKernelResults)."""
    nc = _get_nc(mm_dtype)
    in_maps = _shard(x, y)
    res = run_bass_kernel_spmd(nc, in_maps, core_ids=list(range(B)), trace=trace)
    out = np.stack([res.results[b]["out"].astype(np.float32) for b in range(B)])
    return out, res


def kernel(x: np.ndarray, y: np.ndarray) -> np.ndarray:
    out, _ = _run(x, y)
    return out
